# revision 16
# baseline (speedup 1.0000x reference)
"""CenterPool Trainium2 kernel.

Reference semantics (per bbox):
    img_xc = x + floor(w/2); img_yc = y + floor(h/2)
    cell_x = clip(floor(img_xc/8), 0, 63); cell_y likewise (cell=8px, fm 64x64)
    fv     = input[img_idx, :, cell_y, cell_x]                  # [*, 256]
    label  = [img_xc/8 - cell_x, img_yc/8 - cell_y, w/512, h/512]
    out    = fv + label @ W.T + b

Sharding: data-parallel over batch B=8 across 8 cores (one program, SPMD).
Core b receives input[4b:4b+4] (4 images, 16 MiB) and bboxes[b] (64 boxes);
the 4->256 linear weights are replicated, pre-packed on host as
Wb = [W.T; b] (5,256) so the bias rides the matmul via a ones column.

The gather reads only the 64 KiB actually needed per core (64 boxes x 256
chans x 4 B) instead of streaming the 16 MiB shard. The channel walk is a
16 KiB-strided 256-tap pattern whose base depends on the bbox, which no
Trainium gather primitive expresses (DMA-gather HW offers one offset per
partition with contiguous payload). Instead the kernel computes the 64
flat base offsets on device (exact IEEE floor via the 2^23 round-magic +
is_gt correction), loads each into an engine register, and issues one
register-offset strided DMA per box: dest [2 partitions, 128] at rows
(b, b+64) so descriptors cover all 128 partitions -> all 16 SDMA engines.
Issue alternates between the SP and ACT hardware-DGE rings.

Labels are computed in component-major [1,64] tiles (compute-engine APs
must start 32-aligned, so no partition slicing), multiplied against the
packed weights with accumulating K=1 PE matmuls into a [128,128] PSUM
laid out to match the gathered features, added on DVE, and stored as two
32 KiB DMAs.
"""

import sys

import numpy as np

sys.path.insert(0, "/opt/trn_rl_repo")

from concourse import bacc, bass, mybir, tile  # noqa: E402
from concourse import bass_utils  # noqa: E402

B, K, N, C = 8, 4, 16, 256
FM = 64
HW = FM * FM  # 4096 elements per channel plane
NBOX = K * N  # 64 boxes per core
NCORES = 8
CH = C // 2  # channels per dest row (two rows per box)
MAGIC = 8388608.0  # 2^23: (v + MAGIC) - MAGIC rounds f32 to nearest int
MAXBASE = (K - 1) * C * HW + (FM - 1) * FM + FM - 1  # 3149951

_CACHE = {}  # repeat -> compiled program (input-agnostic)


def _emit_floor(nc, pool, out_ap, v_ap, shape, tag):
    """out = floor(v) for v >= 0, bit-exact IEEE f32 (no HW floor op)."""
    r = pool.tile(shape, mybir.dt.float32, tag=f"flr_r{tag}")
    m = pool.tile(shape, mybir.dt.float32, tag=f"flr_m{tag}")
    nc.vector.tensor_scalar(
        out=r[:], in0=v_ap, scalar1=MAGIC, scalar2=MAGIC,
        op0=mybir.AluOpType.add, op1=mybir.AluOpType.subtract,
    )
    nc.vector.tensor_tensor(out=m[:], in0=r[:], in1=v_ap, op=mybir.AluOpType.is_gt)
    nc.vector.tensor_tensor(out=out_ap, in0=r[:], in1=m[:], op=mybir.AluOpType.subtract)


def _build_program(repeat):
    nc = bacc.Bacc("TRN2", num_devices=NCORES, debug=False, enable_asserts=False)

    inp = nc.dram_tensor("inp", [K, C, FM, FM], mybir.dt.float32, kind="ExternalInput")
    bb_d = nc.dram_tensor("bb", [NBOX, 4], mybir.dt.float32, kind="ExternalInput")
    wb_d = nc.dram_tensor("wb", [5, C], mybir.dt.float32, kind="ExternalInput")
    out_d = nc.dram_tensor("out", [NBOX, C], mybir.dt.float32, kind="ExternalOutput")

    f32 = mybir.dt.float32
    i32 = mybir.dt.int32

    # strided gather view: one dynamic element-offset + uniform 256-tap
    # channel walk (stride 4096 elements). The dest AP splits the walk onto
    # two partitions (b, b+64) x 128 taps. Last AP dim must be contiguous.
    view = bass.AP(tensor=inp, offset=0,
                   ap=[[1, MAXBASE + 1], [HW, C], [1, 1]])

    # register pools for the dynamic offsets (reused round-robin; Tile
    # tracks register def/use ordering)
    regs = {
        "sync": [nc.alloc_register(nc.sync.engine, f"rs{i}") for i in range(8)],
        "scalar": [nc.alloc_register(nc.scalar.engine, f"ra{i}") for i in range(8)],
    }

    with tile.TileContext(nc) as tc:
        with tc.tile_pool(name="p", bufs=2) as pool, \
             tc.tile_pool(name="ps", bufs=2, space="PSUM") as psum_pool:
            for it in range(repeat):
                # ---- loads: bbox components as [1, 64] rows ------------
                comp = []
                for j in range(4):
                    t = pool.tile([1, NBOX], f32, name=f"comp{j}", tag=f"comp{j}")
                    nc.sync.dma_start(
                        out=t[:], in_=bb_d.ap()[:, j:j + 1].rearrange("n f -> f n"))
                    comp.append(t)
                xr, yr, wr, hr = comp
                wbrow = []
                for j in range(5):
                    t = pool.tile([1, C], f32, name=f"wbr{j}", tag=f"wbr{j}")
                    nc.sync.dma_start(out=t[:], in_=wb_d.ap()[j:j + 1, :])
                    wbrow.append(t)

                # ---- cells + labels in [1, 64] component rows ----------
                shp = [1, NBOX]
                halfw = pool.tile(shp, f32)
                vx = pool.tile(shp, f32)
                nc.vector.tensor_scalar_mul(out=vx[:], in0=wr[:], scalar1=0.5)
                _emit_floor(nc, pool, halfw[:], vx[:], shp, "hw")
                halfh = pool.tile(shp, f32)
                vy = pool.tile(shp, f32)
                nc.vector.tensor_scalar_mul(out=vy[:], in0=hr[:], scalar1=0.5)
                _emit_floor(nc, pool, halfh[:], vy[:], shp, "hh")

                v8x = pool.tile(shp, f32)
                nc.vector.tensor_tensor(out=v8x[:], in0=xr[:], in1=halfw[:],
                                        op=mybir.AluOpType.add)
                nc.vector.tensor_scalar_mul(out=v8x[:], in0=v8x[:], scalar1=0.125)
                v8y = pool.tile(shp, f32)
                nc.vector.tensor_tensor(out=v8y[:], in0=yr[:], in1=halfh[:],
                                        op=mybir.AluOpType.add)
                nc.vector.tensor_scalar_mul(out=v8y[:], in0=v8y[:], scalar1=0.125)

                cx = pool.tile(shp, f32)
                _emit_floor(nc, pool, cx[:], v8x[:], shp, "cx")
                nc.vector.tensor_scalar(
                    out=cx[:], in0=cx[:], scalar1=0.0, scalar2=float(FM - 1),
                    op0=mybir.AluOpType.max, op1=mybir.AluOpType.min)
                cy = pool.tile(shp, f32)
                _emit_floor(nc, pool, cy[:], v8y[:], shp, "cy")
                nc.vector.tensor_scalar(
                    out=cy[:], in0=cy[:], scalar1=0.0, scalar2=float(FM - 1),
                    op0=mybir.AluOpType.max, op1=mybir.AluOpType.min)

                fracx = pool.tile(shp, f32)
                nc.vector.tensor_tensor(out=fracx[:], in0=v8x[:], in1=cx[:],
                                        op=mybir.AluOpType.subtract)
                fracy = pool.tile(shp, f32)
                nc.vector.tensor_tensor(out=fracy[:], in0=v8y[:], in1=cy[:],
                                        op=mybir.AluOpType.subtract)
                wn = pool.tile(shp, f32)
                nc.vector.tensor_scalar_mul(out=wn[:], in0=wr[:], scalar1=1.0 / 512.0)
                hn = pool.tile(shp, f32)
                nc.vector.tensor_scalar_mul(out=hn[:], in0=hr[:], scalar1=1.0 / 512.0)
                ones = pool.tile(shp, f32)
                nc.vector.memset(ones[:], 1.0)

                # ---- flat base offsets: k*2^20 + cy*64 + cx ------------
                kbase = pool.tile(shp, i32)
                nc.gpsimd.iota(kbase[:], pattern=[[1, K], [0, N]], base=0,
                               channel_multiplier=0)
                nc.vector.tensor_scalar(
                    out=kbase[:], in0=kbase[:], scalar1=20, scalar2=None,
                    op0=mybir.AluOpType.logical_shift_left)
                base_f = pool.tile(shp, f32)
                nc.vector.tensor_scalar(
                    out=base_f[:], in0=cy[:], scalar1=float(FM), scalar2=None,
                    op0=mybir.AluOpType.mult)
                nc.vector.tensor_tensor(out=base_f[:], in0=base_f[:], in1=cx[:],
                                        op=mybir.AluOpType.add)
                base_i = pool.tile(shp, i32)
                nc.vector.tensor_tensor(out=base_i[:], in0=kbase[:], in1=base_f[:],
                                        op=mybir.AluOpType.add)

                # ---- gather: one register-offset DMA per box -----------
                fv2 = pool.tile([2 * NBOX, CH], f32)
                nc.vector.memset(fv2[:], 0.0)
                for b in range(NBOX):
                    eng, rpool = ((nc.sync, regs["sync"]) if b % 2 == 0
                                  else (nc.scalar, regs["scalar"]))
                    reg = rpool[(b // 2) % len(rpool)]
                    eng.reg_load(reg, base_i[0:1, b:b + 1])
                    sv = nc.snap(reg, donate=True, min_val=0, max_val=MAXBASE)
                    eng.dma_start(out=fv2[b::NBOX, :],
                                  in_=view[bass.ds(sv, 1), :, :])

                # ---- linear: acc2 layout matches fv2 -------------------
                acc2 = psum_pool.tile([2 * NBOX, CH], f32, space="PSUM")
                rows = [fracx, fracy, wn, hn, ones]
                for half in range(2):
                    cs = slice(half * CH, (half + 1) * CH)
                    o = acc2[half * NBOX:(half + 1) * NBOX, :]
                    for j in range(5):
                        nc.tensor.matmul(out=o, lhsT=rows[j][:],
                                         rhs=wbrow[j][:, cs],
                                         start=(j == 0), stop=(j == 4))

                outt = pool.tile([2 * NBOX, CH], f32)
                nc.vector.tensor_tensor(out=outt[:], in0=fv2[:], in1=acc2[:],
                                        op=mybir.AluOpType.add)
                nc.sync.dma_start(out=out_d.ap()[:, 0:CH], in_=outt[0:NBOX, :])
                nc.scalar.dma_start(out=out_d.ap()[:, CH:C],
                                    in_=outt[NBOX:2 * NBOX, :])

    nc.compile()
    return nc


def _get_compiled(repeat=1):
    if repeat not in _CACHE:
        _CACHE[repeat] = _build_program(repeat)
    return _CACHE[repeat]


def _make_in_maps(input, bboxes, W, b):
    wb = np.ascontiguousarray(
        np.concatenate([np.asarray(W, np.float32).T,
                        np.asarray(b, np.float32)[None, :]], axis=0))
    inp = np.asarray(input, np.float32)
    bbx = np.asarray(bboxes, np.float32)
    in_maps = []
    for core in range(NCORES):
        in_maps.append({
            "inp": np.ascontiguousarray(inp[core * K:(core + 1) * K]),
            "bb": np.ascontiguousarray(bbx[core].reshape(NBOX, 4)),
            "wb": wb,
        })
    return in_maps


def run(input, bboxes, W, b, trace=False, repeat=1):
    """Returns (full_output [B,K,N,C] f32, BassKernelResults)."""
    nc = _get_compiled(repeat)
    res = bass_utils.run_bass_kernel_spmd(
        nc, _make_in_maps(input, bboxes, W, b),
        core_ids=list(range(NCORES)), trace=trace,
    )
    out = np.stack([r["out"] for r in res.results], axis=0)  # [8, 64, 256]
    return out.reshape(B, K, N, C), res


def kernel(input, bboxes, W, b):
    out, _ = run(input, bboxes, W, b, trace=False)
    return out
